# revision 6
# baseline (speedup 1.0000x reference)
"""AlternatingHighwayLSTM Trainium2 (Bass) kernel - 8-core SPMD, v2.

Fully transposed recurrence: hidden dim on partitions (4 chunks of 128),
batch (8/core) on the free dim.  Per step, z^T = Wx^T x + Wh^T h + b is
accumulated in PSUM as 24 (128, 8) chunk-slices: Wx/bias parts are
pre-accumulated per 8-step group (double-buffered 3-bank PSUM tiles),
the Wh part runs per step as 80 small bf16 matmuls (stationary = natural
weight chunks).  Sigmoid gates are computed as tanh of half-scaled
pre-activations (the 0.5 is folded into the weights host-side), so one
tanh activation covers i,f,g and one covers o,r.  The elementwise chain
uses fused scalar_tensor_tensor ops on (128, 32) tiles.

Host-side prep: weights/x are cast to bf16, chunked, and column-scaled;
everything is passed as extra DRAM inputs.

Note: like the previous version, the stored per-step state equals h*m
(exact h for the eval-mode all-ones dropout mask).
"""
import numpy as np
import ml_dtypes
import concourse.bass as bass
import concourse.mybir as mybir
import concourse.tile as tile
from concourse.bass_utils import run_bass_kernel_spmd
from concourse.masks import make_identity

F32 = mybir.dt.float32
F32R = mybir.dt.float32r
BF16 = mybir.dt.bfloat16
AF = mybir.ActivationFunctionType
OP = mybir.AluOpType

H = 512
B = 8            # per-core batch
G = 8            # steps per Wx pre-accumulation group
NBF16 = ml_dtypes.bfloat16

_DMA_OPS = {"DMACopy", "TensorLoad", "TensorSave", "DMATransposeAnt", "TriggerDMA"}


def split_excess_waits(nc, limit=1, dma_limit=1):
    n_split = 0
    uid = [0]
    for f in nc.m.functions:
        for b in f.blocks:
            out = []
            changed = False
            for ins in b.instructions:
                si = ins.sync_info
                lim = dma_limit if ins.opcode in _DMA_OPS else limit
                if si is not None and si.on_wait is not None and len(si.on_wait) > lim:
                    waits = list(si.on_wait)
                    extra, keep = waits[:-lim], waits[-lim:]
                    for w in extra:
                        ev = mybir.InstEventSemaphore(name=f"WSPLIT-{uid[0]}")
                        uid[0] += 1
                        ev.engine = ins.engine
                        ev.sync_info = mybir.SyncInfo(on_wait=[w], on_update=[])
                        out.append(ev)
                        n_split += 1
                    ins.sync_info = mybir.SyncInfo(
                        on_wait=keep, on_update=list(si.on_update or []))
                    changed = True
                out.append(ins)
            if changed:
                try:
                    b.instructions = out
                except Exception:
                    b.instructions.clear()
                    b.instructions.extend(out)
    return n_split


def build_full(S=256, L=8, dbg_hseq=False, wx_first=False, use_gpsimd=True):
    nc = bass.Bass("TRN2", target_bir_lowering=False, debug=False)
    T = S * B          # free-dim cols per hidden chunk (tokens x batch)
    NG = S // G
    dbg_d = None
    if dbg_hseq:
        dbg_d = nc.dram_tensor("dbg", [L, 128, 4 * T], BF16,
                               kind="ExternalOutput").ap()

    xt0_d = nc.dram_tensor("xt0", [128, 8 * T], BF16, kind="ExternalInput").ap()
    wx0_d = nc.dram_tensor("wx0", [128, 8 * 3072], BF16, kind="ExternalInput").ap()
    wxr_d = nc.dram_tensor("wxr", [max(L - 1, 1), 128, 4 * 3072], BF16,
                           kind="ExternalInput").ap()
    wh_d = nc.dram_tensor("whx", [L, 128, 4 * 2560], BF16,
                          kind="ExternalInput").ap()
    bias_d = nc.dram_tensor("biasx", [L, 2560], BF16, kind="ExternalInput").ap()
    mask_d = nc.dram_tensor("maskrep", [L, 128, 256], F32,
                            kind="ExternalInput").ap()
    # output layout (S, 4 chunks x 8 batch, 128): matches the PE-transpose
    # partition order; the host unshuffles to (S, B, H)
    o_d = nc.dram_tensor("out", [S, 32, 128], F32, kind="ExternalOutput").ap()

    with tile.TileContext(nc) as tc:
        with (
            tc.tile_pool(name="sb", bufs=1) as sb,
            tc.tile_pool(name="ps", bufs=1, space="PSUM") as ps,
        ):
            xT0 = sb.tile([128, 8 * T], BF16, tag="xT0")
            hseq = [sb.tile([128, 4 * T], BF16, tag=f"hseq{i}", name=f"hseq{i}")
                    for i in range(2)]
            wxs = [sb.tile([128, 4 * 3072], BF16, tag=f"wxs{i}", name=f"wxs{i}")
                   for i in range(2)]
            whs = [sb.tile([128, 4 * 2560], BF16, tag=f"whs{i}", name=f"whs{i}")
                   for i in range(2)]
            biast = [sb.tile([1, 2560], BF16, tag=f"bias{i}", name=f"bias{i}")
                     for i in range(2)]
            maskt = [sb.tile([128, 256], F32, tag=f"mask{i}", name=f"mask{i}")
                     for i in range(2)]
            ones_t = sb.tile([1, 64], BF16, tag="ones")
            gates = sb.tile([128, 192], F32, tag="gates")
            scr = sb.tile([128, 64], F32, tag="scr")
            scr2 = sb.tile([128, 64], F32, tag="scr2")
            thb = sb.tile([128, 32], F32, tag="thb")
            tln = sb.tile([128, 32], F32, tag="tln")
            Xt = sb.tile([128, 32], F32, tag="Xt")
            lin05 = [sb.tile([128, 256], F32, tag=f"lin05{i}", name=f"lin05{i}")
                     for i in range(2)]
            hm32 = sb.tile([128, 32], F32, tag="hm32")
            ob = [sb.tile([32, 128], F32, tag=f"ob{i}", name=f"ob{i}")
                  for i in range(2)]
            ident = sb.tile([128, 128], F32, tag="ident")

            zg = [ps.tile([128, 1536], F32, tag=f"zg{i}", name=f"zg{i}")
                  for i in range(2)]
            tp = [ps.tile([128, 512], F32, tag=f"tp{i}", name=f"tp{i}")
                  for i in range(2)]

            make_identity(nc, ident[:, :])
            nc.vector.memset(ones_t[0:1, :], 1.0)

            # initial loads
            nc.sync.dma_start(out=xT0[:, :], in_=xt0_d)
            nc.sync.dma_start(out=wxs[0][:, :], in_=wx0_d[:, 0:12288])
            nc.sync.dma_start(out=wxs[1][:, :], in_=wx0_d[:, 12288:24576])
            nc.sync.dma_start(out=whs[0][:, :], in_=wh_d[0])
            nc.sync.dma_start(out=biast[0][0:1, :],
                              in_=bias_d[0].rearrange("(o c) -> o c", o=1))
            nc.sync.dma_start(out=maskt[0][:, :], in_=mask_d[0])

            for l in range(L):
                even = (l % 2 == 0)
                ind = l % 2
                xin = xT0 if l == 0 else hseq[(l - 1) % 2]
                KCX = 8 if l == 0 else 4
                whv = whs[l % 2]
                bv = biast[l % 2]
                mkt = maskt[l % 2]

                def wxv(k, l=l):
                    if l == 0:
                        return wxs[k // 4][:, (k % 4) * 3072:(k % 4 + 1) * 3072]
                    return wxs[(l + 1) % 2][:, k * 3072:(k + 1) * 3072]

                def tok(s, even=even):
                    return s if even else S - 1 - s

                def tokbase(g, even=even):
                    return g * G if even else S - (g + 1) * G

                def wx_group_thunks(g, xin=xin, KCX=KCX, wxv=wxv,
                                    tokbase=tokbase, bv=bv):
                    z = zg[g % 2]
                    tb = tokbase(g)
                    items = [("b", 0), ("b", 8), ("b", 16)]
                    for c in range(24):
                        if c < 20 and c not in (0, 8, 16):
                            items.append(("b", c))
                        for k in range(KCX):
                            items.append(("x", c, k))
                    first = {0: True, 1: True, 2: True}
                    thunks = []
                    for it in items:
                        bank = it[1] // 8
                        st = first[bank]
                        first[bank] = False
                        if it[0] == "b":
                            c = it[1]

                            def t(c=c, st=st, z=z):
                                nc.tensor.matmul(
                                    z[:, c * 64:(c + 1) * 64],
                                    bv[0:1, c * 128:(c + 1) * 128],
                                    ones_t[0:1, 0:64],
                                    start=st, stop=False, skip_group_check=True)
                        else:
                            c, k = it[1], it[2]

                            def t(c=c, k=k, st=st, z=z, tb=tb):
                                nc.tensor.matmul(
                                    z[:, c * 64:(c + 1) * 64],
                                    wxv(k)[:, c * 128:(c + 1) * 128],
                                    xin[:, k * T + tb * 8:k * T + (tb + G) * 8],
                                    start=st, stop=False, skip_group_check=True)
                        thunks.append(t)
                    return thunks

                # group 0 burst
                for t in wx_group_thunks(0):
                    t()

                # prefetch next layer's weights (the wx slot for layer l+1 is
                # only free of emitted readers once layer l-1 is emitted; for
                # l==0 both slots are read by layer 0 itself, so its wx
                # prefetch is emitted after the step loop below)
                if l + 1 < L:
                    if l > 0:
                        nc.sync.dma_start(out=wxs[(l + 2) % 2][:, :],
                                          in_=wxr_d[l])
                    nc.sync.dma_start(out=whs[(l + 1) % 2][:, :], in_=wh_d[l + 1])
                    nc.sync.dma_start(
                        out=biast[(l + 1) % 2][0:1, :],
                        in_=bias_d[l + 1].rearrange("(o c) -> o c", o=1))
                    nc.sync.dma_start(out=maskt[(l + 1) % 2][:, :],
                                      in_=mask_d[l + 1])

                nc.vector.memset(gates[:, 96:128], 0.0)  # c-hat = 0

                pending = []
                for s in range(S):
                    g, j = divmod(s, G)
                    z = zg[g % 2]
                    z3 = z.rearrange("p (c x) -> p c x", c=24)
                    pos = j if even else G - 1 - j
                    zoff = pos * 8

                    if j == 0:
                        if g + 1 < NG:
                            pending = wx_group_thunks(g + 1)
                        else:
                            pending = []
                        # lin05m = 0.5 * lin * mask  (whole group at once)
                        nc.vector.scalar_tensor_tensor(
                            out=lin05[g % 2][:, :], in0=z[:, 1280:1536],
                            scalar=0.5, in1=mkt[:, :],
                            op0=OP.mult, op1=OP.mult)

                    # optionally emit next group's Wx slice before the Wh
                    # burst (they have no step-chain deps)
                    if wx_first and pending:
                        nsl = (len(pending) + (G - 1 - j)) // (G - j)
                        for t in pending[:nsl]:
                            t()
                        pending = pending[nsl:]

                    # Wh matmuls for this step
                    if s > 0:
                        tp_ = tok(s - 1)
                        for c in range(20):
                            for k in range(4):
                                stop = (j == G - 1 and k == 3
                                        and c in (7, 15, 19))
                                nc.tensor.matmul(
                                    z[:, c * 64 + zoff:c * 64 + zoff + 8],
                                    whv[:, k * 2560 + c * 128:
                                        k * 2560 + (c + 1) * 128],
                                    hseq[ind][:, k * T + tp_ * 8:
                                              k * T + tp_ * 8 + 8],
                                    start=False, stop=stop,
                                    skip_group_check=True)

                    # activations: tanh(z') with 0.5 folded into weights for
                    # sigmoid gates -> [ti tf tg | to tr]
                    nc.scalar.activation(
                        gates[:, 0:96].rearrange("p (c x) -> p c x", c=12),
                        z3[:, 0:12, zoff:zoff + 8], AF.Tanh)
                    nc.scalar.activation(
                        gates[:, 128:192].rearrange("p (c x) -> p c x", c=8),
                        z3[:, 12:20, zoff:zoff + 8], AF.Tanh)

                    # [t2|u2] = ([ti|tf] + 1) * [tg|chat]
                    nc.vector.scalar_tensor_tensor(
                        out=scr[:, 0:64], in0=gates[:, 0:64], scalar=1.0,
                        in1=gates[:, 64:128], op0=OP.add, op1=OP.mult)
                    # chat' = 0.5*u2 + t2   (chat = 2c)
                    nc.vector.scalar_tensor_tensor(
                        out=gates[:, 96:128], in0=scr[:, 32:64], scalar=0.5,
                        in1=scr[:, 0:32], op0=OP.mult, op1=OP.add)

                    l5v = lin05[g % 2].rearrange("p (c x) -> p c x", c=4)[
                        :, :, zoff:zoff + 8]
                    trv = gates[:, 160:192].rearrange("p (c x) -> p c x", c=4)
                    # tlnm = tr * lin05m ; X = lin05m - tlnm = (1-r)*lin*m
                    # (Pool supports TensorTensor but not TensorScalarPtr)
                    veng = nc.gpsimd if use_gpsimd else nc.vector
                    veng.tensor_tensor(
                        tln[:, :].rearrange("p (c x) -> p c x", c=4),
                        trv, l5v, OP.mult)
                    nc.vector.scalar_tensor_tensor(
                        out=Xt[:, :].rearrange("p (c x) -> p c x", c=4),
                        in0=tln[:, :].rearrange("p (c x) -> p c x", c=4),
                        scalar=-1.0, in1=l5v, op0=OP.mult, op1=OP.add)

                    # th = tanh(c) = tanh(0.5 * chat)
                    nc.scalar.activation(thb[:, :], gates[:, 96:128], AF.Tanh,
                                         scale=0.5)
                    # q2 = (to + 1) * th ; w4 = (tr + 1) * q2 = 4*r*o*th
                    nc.vector.scalar_tensor_tensor(
                        out=scr2[:, 0:32], in0=gates[:, 128:160], scalar=1.0,
                        in1=thb[:, :], op0=OP.add, op1=OP.mult)
                    nc.vector.scalar_tensor_tensor(
                        out=scr2[:, 32:64], in0=gates[:, 160:192], scalar=1.0,
                        in1=scr2[:, 0:32], op0=OP.add, op1=OP.mult)
                    # hm = 0.25*w4 + X  -> hseq (bf16, strided chunk view)
                    hv = hseq[ind].rearrange("p (k t) -> p k t", k=4)[
                        :, :, tok(s) * 8:tok(s) * 8 + 8]
                    nc.vector.scalar_tensor_tensor(
                        out=hv, in0=scr2[:, 32:64].rearrange(
                            "p (c x) -> p c x", c=4),
                        scalar=0.25,
                        in1=Xt[:, :].rearrange("p (c x) -> p c x", c=4),
                        op0=OP.mult, op1=OP.add)

                    if l == L - 1:
                        nc.vector.scalar_tensor_tensor(
                            out=hm32[:, :], in0=scr2[:, 32:64], scalar=0.25,
                            in1=Xt[:, :], op0=OP.mult, op1=OP.add)
                        nc.tensor.transpose(
                            tp[s % 2][0:32, 0:128],
                            hm32[:, 0:32], ident[:, :])
                        nc.scalar.copy(ob[s % 2][0:32, :],
                                       tp[s % 2][0:32, 0:128])
                        nc.sync.dma_start(out=o_d[tok(s), :, :],
                                          in_=ob[s % 2][0:32, :])

                    if dbg_hseq and s == S - 1:
                        nc.sync.dma_start(out=dbg_d[l], in_=hseq[ind][:, :])

                    # spread next group's Wx/bias matmuls
                    if pending and not wx_first:
                        nsl = (len(pending) + (G - 1 - j)) // (G - j)
                        for t in pending[:nsl]:
                            t()
                        pending = pending[nsl:]

                if l == 0 and L > 1:
                    nc.sync.dma_start(out=wxs[0][:, :], in_=wxr_d[0])
    return nc


def strip_self_waits(nc):
    """Remove waits on an instruction's own engine-completion semaphore.

    DVE/Activation/PE engine queues execute strictly in order, so a
    same-engine RAW/WAR dependency is already satisfied by program order;
    the tile framework still emits a self-semaphore wait for it, which
    costs the producer's full completion-propagation latency on the
    consumer.  Pool (8 concurrent Q7 cores) and DMA instructions keep all
    waits.
    """
    import re
    eng_prefix = {
        mybir.EngineType.DVE: "DVE_",
        mybir.EngineType.Activation: "Activation_",
        mybir.EngineType.PE: "PE_",
    }
    n_strip = 0
    for f in nc.m.functions:
        for b in f.blocks:
            for ins in b.instructions:
                if ins.opcode in _DMA_OPS:
                    continue
                pref = eng_prefix.get(ins.engine)
                si = ins.sync_info
                if pref is None or si is None or not si.on_wait:
                    continue
                keep = [w for w in si.on_wait
                        if not (w.ant_name or "").startswith(pref)]
                if len(keep) != len(si.on_wait):
                    n_strip += len(si.on_wait) - len(keep)
                    ins.sync_info = mybir.SyncInfo(
                        on_wait=keep, on_update=list(si.on_update or []))
    return n_strip


# ---- host-side input prep ----


def _prep_mask(mask_slice, L=8):
    # mask_slice: (L, 8, 512) -> (L, 128, 256) transposed, token-repeated
    mr = np.empty((L, 128, 256), np.float32)
    for l in range(L):
        mT = mask_slice[l].T.reshape(4, 128, B).transpose(1, 0, 2)  # p,c,b
        mr[l] = np.broadcast_to(
            mT[:, :, None, :], (128, 4, 8, B)).reshape(128, 256)
    return mr


def _prep_shared(weight, bias, L=8, IN=1024):
    w_off = 0
    wx_list, wh_list, b_list = [], [], []
    for l in range(L):
        in_l = IN if l == 0 else H
        wx = weight[w_off:w_off + in_l * 3072].reshape(in_l, 3072).copy()
        w_off += in_l * 3072
        wh = weight[w_off:w_off + H * 2560].reshape(H, 2560).copy()
        w_off += H * 2560
        b = bias[l * 2560:(l + 1) * 2560].copy()
        # fold sigmoid(z) = 0.5*(1+tanh(z/2)) halving into i,f,o,r columns
        sc6 = np.ones(3072, np.float32)
        sc6[0:1024] = 0.5          # i, f
        sc6[1536:2560] = 0.5       # o, r
        wx *= sc6[None, :]
        wh *= sc6[None, :2560]
        b = b * sc6[:2560]
        wx_list.append(wx)
        wh_list.append(wh)
        b_list.append(b)

    wx0 = wx_list[0].reshape(8, 128, 3072).transpose(1, 0, 2).reshape(
        128, 8 * 3072).astype(NBF16)
    wxr = np.stack([
        wx_list[l].reshape(4, 128, 3072).transpose(1, 0, 2).reshape(
            128, 4 * 3072) for l in range(1, L)]).astype(NBF16)
    wh = np.stack([
        wh_list[l].reshape(4, 128, 2560).transpose(1, 0, 2).reshape(
            128, 4 * 2560) for l in range(L)]).astype(NBF16)
    bb = np.stack(b_list).astype(NBF16)
    return wx0, wxr, wh, bb


def _prep_x(x_slice, S):
    # x_slice: (S, B, 1024) -> (128, 8 chunks * S * B) bf16
    xt = x_slice.transpose(2, 0, 1).reshape(8, 128, S * B)
    return np.ascontiguousarray(
        xt.transpose(1, 0, 2).reshape(128, 8 * S * B)).astype(NBF16)


_CACHE = {}


def _get_nc():
    if "nc" not in _CACHE:
        nc = build_full(S=256, L=8)
        strip_self_waits(nc)
        split_excess_waits(nc)
        _CACHE["nc"] = nc
    return _CACHE["nc"]


def hw_exec_time_estimate_ns():
    if "tl" not in _CACHE:
        from concourse.timeline_sim import TimelineSim
        _CACHE["tl"] = int(TimelineSim(_get_nc(), trace=False).simulate())
    return _CACHE["tl"]


def kernel(inputs, weight, bias, dropout_mask):
    inputs = np.ascontiguousarray(inputs, dtype=np.float32)
    weight = np.ascontiguousarray(weight, dtype=np.float32)
    bias = np.ascontiguousarray(bias, dtype=np.float32)
    dropout_mask = np.ascontiguousarray(dropout_mask, dtype=np.float32)
    S = inputs.shape[0]
    nc = _get_nc()
    wx0, wxr, wh, bb = _prep_shared(weight, bias)
    n_cores = 8
    in_maps = []
    for i in range(n_cores):
        sl = slice(B * i, B * (i + 1))
        in_maps.append({
            "xt0": _prep_x(inputs[:, sl, :], S),
            "wx0": wx0, "wxr": wxr, "whx": wh, "biasx": bb,
            "maskrep": _prep_mask(dropout_mask[:, sl, :]),
        })
    res = run_bass_kernel_spmd(nc, in_maps, list(range(n_cores)))
    outs = []
    for i in range(n_cores):
        o = res.results[i]["out"].reshape(S, 4, B, 128)
        outs.append(o.transpose(0, 2, 1, 3).reshape(S, B, H))
    out = np.concatenate(outs, axis=1)
    return np.ascontiguousarray(out, dtype=np.float32)


# revision 7
# speedup vs baseline: 1.0657x; 1.0657x over previous
"""AlternatingHighwayLSTM Trainium2 (Bass) kernel - 8-core SPMD, v2.

Fully transposed recurrence: hidden dim on partitions (4 chunks of 128),
batch (8/core) on the free dim.  Per step, z^T = Wx^T x + Wh^T h + b is
accumulated in PSUM as 24 (128, 8) chunk-slices: Wx/bias parts are
pre-accumulated per 8-step group (double-buffered 3-bank PSUM tiles),
the Wh part runs per step as 80 small bf16 matmuls (stationary = natural
weight chunks).  Sigmoid gates are computed as tanh of half-scaled
pre-activations (the 0.5 is folded into the weights host-side), so one
tanh activation covers i,f,g and one covers o,r.  The elementwise chain
uses fused scalar_tensor_tensor ops on (128, 32) tiles.

Host-side prep: weights/x are cast to bf16, chunked, and column-scaled;
everything is passed as extra DRAM inputs.

Note: like the previous version, the stored per-step state equals h*m
(exact h for the eval-mode all-ones dropout mask).
"""
import numpy as np
import ml_dtypes
import concourse.bass as bass
import concourse.mybir as mybir
import concourse.tile as tile
from concourse.bass_utils import run_bass_kernel_spmd
from concourse.masks import make_identity

F32 = mybir.dt.float32
F32R = mybir.dt.float32r
BF16 = mybir.dt.bfloat16
AF = mybir.ActivationFunctionType
OP = mybir.AluOpType

H = 512
B = 8            # per-core batch
G = 8            # steps per Wx pre-accumulation group
NBF16 = ml_dtypes.bfloat16

_DMA_OPS = {"DMACopy", "TensorLoad", "TensorSave", "DMATransposeAnt", "TriggerDMA"}


def split_excess_waits(nc, limit=1, dma_limit=1):
    n_split = 0
    uid = [0]
    for f in nc.m.functions:
        for b in f.blocks:
            out = []
            changed = False
            for ins in b.instructions:
                si = ins.sync_info
                lim = dma_limit if ins.opcode in _DMA_OPS else limit
                if si is not None and si.on_wait is not None and len(si.on_wait) > lim:
                    waits = list(si.on_wait)
                    extra, keep = waits[:-lim], waits[-lim:]
                    for w in extra:
                        ev = mybir.InstEventSemaphore(name=f"WSPLIT-{uid[0]}")
                        uid[0] += 1
                        ev.engine = ins.engine
                        ev.sync_info = mybir.SyncInfo(on_wait=[w], on_update=[])
                        out.append(ev)
                        n_split += 1
                    ins.sync_info = mybir.SyncInfo(
                        on_wait=keep, on_update=list(si.on_update or []))
                    changed = True
                out.append(ins)
            if changed:
                try:
                    b.instructions = out
                except Exception:
                    b.instructions.clear()
                    b.instructions.extend(out)
    return n_split


def build_full(S=256, L=8, dbg_hseq=False, wx_first=False, use_gpsimd=True):
    nc = bass.Bass("TRN2", target_bir_lowering=False, debug=False)
    T = S * B          # free-dim cols per hidden chunk (tokens x batch)
    NG = S // G
    dbg_d = None
    if dbg_hseq:
        dbg_d = nc.dram_tensor("dbg", [L, 128, 4 * T], BF16,
                               kind="ExternalOutput").ap()

    xt0_d = nc.dram_tensor("xt0", [128, 8 * T], BF16, kind="ExternalInput").ap()
    wx0_d = nc.dram_tensor("wx0", [128, 8 * 3072], BF16, kind="ExternalInput").ap()
    wxr_d = nc.dram_tensor("wxr", [max(L - 1, 1), 128, 4 * 3072], BF16,
                           kind="ExternalInput").ap()
    wh_d = nc.dram_tensor("whx", [L, 128, 4 * 2560], BF16,
                          kind="ExternalInput").ap()
    bias_d = nc.dram_tensor("biasx", [L, 2560], BF16, kind="ExternalInput").ap()
    mask_d = nc.dram_tensor("maskrep", [L, 128, 256], F32,
                            kind="ExternalInput").ap()
    # output layout (S, 4 chunks x 8 batch, 128): matches the PE-transpose
    # partition order; the host unshuffles to (S, B, H)
    o_d = nc.dram_tensor("out", [S, 32, 128], F32, kind="ExternalOutput").ap()

    with tile.TileContext(nc) as tc:
        with (
            tc.tile_pool(name="sb", bufs=1) as sb,
            tc.tile_pool(name="ps", bufs=1, space="PSUM") as ps,
        ):
            xT0 = sb.tile([128, 8 * T], BF16, tag="xT0")
            hseq = [sb.tile([128, 4 * T], BF16, tag=f"hseq{i}", name=f"hseq{i}")
                    for i in range(2)]
            wxs = [sb.tile([128, 4 * 3072], BF16, tag=f"wxs{i}", name=f"wxs{i}")
                   for i in range(2)]
            whs = [sb.tile([128, 4 * 2560], BF16, tag=f"whs{i}", name=f"whs{i}")
                   for i in range(2)]
            biast = [sb.tile([1, 2560], BF16, tag=f"bias{i}", name=f"bias{i}")
                     for i in range(2)]
            maskt = [sb.tile([128, 256], F32, tag=f"mask{i}", name=f"mask{i}")
                     for i in range(2)]
            ones_t = sb.tile([1, 64], BF16, tag="ones")
            gates = sb.tile([128, 192], F32, tag="gates")
            scr = sb.tile([128, 64], F32, tag="scr")
            scr2 = sb.tile([128, 64], F32, tag="scr2")
            thb = sb.tile([128, 32], F32, tag="thb")
            tln = sb.tile([128, 32], F32, tag="tln")
            Xt = sb.tile([128, 32], F32, tag="Xt")
            lin05 = [sb.tile([128, 256], F32, tag=f"lin05{i}", name=f"lin05{i}")
                     for i in range(2)]
            hm32 = sb.tile([128, 32], F32, tag="hm32")
            b1q = sb.tile([128, 32], F32, tag="b1q")
            p4q = sb.tile([128, 32], F32, tag="p4q")
            wv = sb.tile([128, 32], F32, tag="wv")
            quart = sb.tile([128, 32], F32, tag="quart")
            ob = [sb.tile([32, 128], F32, tag=f"ob{i}", name=f"ob{i}")
                  for i in range(2)]
            ident = sb.tile([128, 128], F32, tag="ident")

            zg = [ps.tile([128, 1536], F32, tag=f"zg{i}", name=f"zg{i}")
                  for i in range(2)]
            tp = [ps.tile([128, 512], F32, tag=f"tp{i}", name=f"tp{i}")
                  for i in range(2)]

            make_identity(nc, ident[:, :])
            nc.vector.memset(ones_t[0:1, :], 1.0)
            nc.vector.memset(quart[:, :], 0.25)

            # initial loads
            nc.sync.dma_start(out=xT0[:, :], in_=xt0_d)
            nc.sync.dma_start(out=wxs[0][:, :], in_=wx0_d[:, 0:12288])
            nc.sync.dma_start(out=wxs[1][:, :], in_=wx0_d[:, 12288:24576])
            nc.sync.dma_start(out=whs[0][:, :], in_=wh_d[0])
            nc.sync.dma_start(out=biast[0][0:1, :],
                              in_=bias_d[0].rearrange("(o c) -> o c", o=1))
            nc.sync.dma_start(out=maskt[0][:, :], in_=mask_d[0])

            for l in range(L):
                even = (l % 2 == 0)
                ind = l % 2
                xin = xT0 if l == 0 else hseq[(l - 1) % 2]
                KCX = 8 if l == 0 else 4
                whv = whs[l % 2]
                bv = biast[l % 2]
                mkt = maskt[l % 2]

                def wxv(k, l=l):
                    if l == 0:
                        return wxs[k // 4][:, (k % 4) * 3072:(k % 4 + 1) * 3072]
                    return wxs[(l + 1) % 2][:, k * 3072:(k + 1) * 3072]

                def tok(s, even=even):
                    return s if even else S - 1 - s

                def tokbase(g, even=even):
                    return g * G if even else S - (g + 1) * G

                def wx_group_thunks(g, xin=xin, KCX=KCX, wxv=wxv,
                                    tokbase=tokbase, bv=bv):
                    z = zg[g % 2]
                    tb = tokbase(g)
                    items = [("b", 0), ("b", 8), ("b", 16)]
                    for c in range(24):
                        if c < 20 and c not in (0, 8, 16):
                            items.append(("b", c))
                        for k in range(KCX):
                            items.append(("x", c, k))
                    first = {0: True, 1: True, 2: True}
                    thunks = []
                    for it in items:
                        bank = it[1] // 8
                        st = first[bank]
                        first[bank] = False
                        if it[0] == "b":
                            c = it[1]

                            def t(c=c, st=st, z=z):
                                nc.tensor.matmul(
                                    z[:, c * 64:(c + 1) * 64],
                                    bv[0:1, c * 128:(c + 1) * 128],
                                    ones_t[0:1, 0:64],
                                    start=st, stop=False, skip_group_check=True)
                        else:
                            c, k = it[1], it[2]

                            def t(c=c, k=k, st=st, z=z, tb=tb):
                                nc.tensor.matmul(
                                    z[:, c * 64:(c + 1) * 64],
                                    wxv(k)[:, c * 128:(c + 1) * 128],
                                    xin[:, k * T + tb * 8:k * T + (tb + G) * 8],
                                    start=st, stop=False, skip_group_check=True)
                        thunks.append(t)
                    return thunks

                # group 0 burst
                for t in wx_group_thunks(0):
                    t()

                # prefetch next layer's weights (the wx slot for layer l+1 is
                # only free of emitted readers once layer l-1 is emitted; for
                # l==0 both slots are read by layer 0 itself, so its wx
                # prefetch is emitted after the step loop below)
                if l + 1 < L:
                    if l > 0:
                        nc.sync.dma_start(out=wxs[(l + 2) % 2][:, :],
                                          in_=wxr_d[l])
                    nc.sync.dma_start(out=whs[(l + 1) % 2][:, :], in_=wh_d[l + 1])
                    nc.sync.dma_start(
                        out=biast[(l + 1) % 2][0:1, :],
                        in_=bias_d[l + 1].rearrange("(o c) -> o c", o=1))
                    nc.sync.dma_start(out=maskt[(l + 1) % 2][:, :],
                                      in_=mask_d[l + 1])

                nc.vector.memset(gates[:, 96:128], 0.0)  # c-hat = 0

                pending = []
                for s in range(S):
                    g, j = divmod(s, G)
                    z = zg[g % 2]
                    z3 = z.rearrange("p (c x) -> p c x", c=24)
                    pos = j if even else G - 1 - j
                    zoff = pos * 8

                    if j == 0:
                        if g + 1 < NG:
                            pending = wx_group_thunks(g + 1)
                        else:
                            pending = []
                        if s == 0:
                            # group 0: lin cols just produced by the burst
                            nc.vector.scalar_tensor_tensor(
                                out=lin05[0][:, :], in0=z[:, 1280:1536],
                                scalar=0.5, in1=mkt[:, :],
                                op0=OP.mult, op1=OP.mult)

                    # optionally emit next group's Wx slice before the Wh
                    # burst (they have no step-chain deps)
                    if wx_first and pending:
                        nsl = (len(pending) + (G - 1 - j)) // (G - j)
                        for t in pending[:nsl]:
                            t()
                        pending = pending[nsl:]

                    # Wh matmuls for this step (rhs = previous step's hm)
                    if s > 0:
                        tp_ = tok(s - 1)
                        for c in range(20):
                            for k in range(4):
                                stop = (j == G - 1 and k == 3
                                        and c in (7, 15, 19))
                                nc.tensor.matmul(
                                    z[:, c * 64 + zoff:c * 64 + zoff + 8],
                                    whv[:, k * 2560 + c * 128:
                                        k * 2560 + (c + 1) * 128],
                                    hseq[ind][:, k * T + tp_ * 8:
                                              k * T + tp_ * 8 + 8],
                                    start=False, stop=stop,
                                    skip_group_check=True)

                    # activations: tanh(z') with 0.5 folded into weights for
                    # sigmoid gates -> [ti tf tg | to tr]
                    nc.scalar.activation(
                        gates[:, 0:96].rearrange("p (c x) -> p c x", c=12),
                        z3[:, 0:12, zoff:zoff + 8], AF.Tanh)
                    nc.scalar.activation(
                        gates[:, 128:192].rearrange("p (c x) -> p c x", c=8),
                        z3[:, 12:20, zoff:zoff + 8], AF.Tanh)

                    # [t2|u2] = ([ti|tf] + 1) * [tg|chat]
                    nc.vector.scalar_tensor_tensor(
                        out=scr[:, 0:64], in0=gates[:, 0:64], scalar=1.0,
                        in1=gates[:, 64:128], op0=OP.add, op1=OP.mult)
                    # chat' = 0.5*u2 + t2   (chat = 2c)
                    nc.vector.scalar_tensor_tensor(
                        out=gates[:, 96:128], in0=scr[:, 32:64], scalar=0.5,
                        in1=scr[:, 0:32], op0=OP.mult, op1=OP.add)

                    l5v = lin05[g % 2].rearrange("p (c x) -> p c x", c=4)[
                        :, :, zoff:zoff + 8]
                    trv = gates[:, 160:192].rearrange("p (c x) -> p c x", c=4)
                    # off-path: p4q = r*o = 0.25*(to+1)*(tr+1)  (queued
                    # before X so the Pool-gated X doesn't delay it)
                    nc.vector.scalar_tensor_tensor(
                        out=b1q[:, :], in0=gates[:, 128:160], scalar=1.0,
                        in1=quart[:, :], op0=OP.add, op1=OP.mult)
                    nc.vector.scalar_tensor_tensor(
                        out=p4q[:, :], in0=gates[:, 160:192], scalar=1.0,
                        in1=b1q[:, :], op0=OP.add, op1=OP.mult)
                    # tlnm = tr * lin05m ; X = lin05m - tlnm = (1-r)*lin*m
                    # (Pool supports TensorTensor but not TensorScalarPtr)
                    veng = nc.gpsimd if use_gpsimd else nc.vector
                    veng.tensor_tensor(
                        tln[:, :].rearrange("p (c x) -> p c x", c=4),
                        trv, l5v, OP.mult)
                    nc.vector.scalar_tensor_tensor(
                        out=Xt[:, :].rearrange("p (c x) -> p c x", c=4),
                        in0=tln[:, :].rearrange("p (c x) -> p c x", c=4),
                        scalar=-1.0, in1=l5v, op0=OP.mult, op1=OP.add)
                    # th = tanh(c) = tanh(0.5 * chat)
                    nc.scalar.activation(thb[:, :], gates[:, 96:128], AF.Tanh,
                                         scale=0.5)
                    # w = r*o*th ; hm = w + X   (short post-tanh path)
                    nc.vector.tensor_tensor(wv[:, :], p4q[:, :], thb[:, :],
                                            OP.mult)
                    # hm -> hseq (bf16, strided chunk view)
                    hv = hseq[ind].rearrange("p (k t) -> p k t", k=4)[
                        :, :, tok(s) * 8:tok(s) * 8 + 8]
                    nc.vector.tensor_tensor(
                        hv, wv[:, :].rearrange("p (c x) -> p c x", c=4),
                        Xt[:, :].rearrange("p (c x) -> p c x", c=4), OP.add)

                    if l == L - 1:
                        nc.vector.tensor_tensor(hm32[:, :], wv[:, :],
                                                Xt[:, :], OP.add)
                        nc.tensor.transpose(
                            tp[s % 2][0:32, 0:128],
                            hm32[:, 0:32], ident[:, :])
                        nc.scalar.copy(ob[s % 2][0:32, :],
                                       tp[s % 2][0:32, 0:128])
                        nc.sync.dma_start(out=o_d[tok(s), :, :],
                                          in_=ob[s % 2][0:32, :])

                    if dbg_hseq and s == S - 1:
                        nc.sync.dma_start(out=dbg_d[l], in_=hseq[ind][:, :])

                    # spread next group's Wx/bias matmuls
                    if pending and not wx_first:
                        nsl = (len(pending) + (G - 1 - j)) // (G - j)
                        for t in pending[:nsl]:
                            t()
                        pending = pending[nsl:]

                    if j == G - 1 and g + 1 < NG:
                        # next group's lin05m at the tail of this step's DVE
                        # queue (after its Wx lin matmuls are all emitted)
                        nc.vector.scalar_tensor_tensor(
                            out=lin05[(g + 1) % 2][:, :],
                            in0=zg[(g + 1) % 2][:, 1280:1536],
                            scalar=0.5, in1=mkt[:, :],
                            op0=OP.mult, op1=OP.mult)

                if l == 0 and L > 1:
                    nc.sync.dma_start(out=wxs[0][:, :], in_=wxr_d[0])
    return nc


def strip_self_waits(nc):
    """Remove waits on an instruction's own engine-completion semaphore.

    DVE/Activation/PE engine queues execute strictly in order, so a
    same-engine RAW/WAR dependency is already satisfied by program order;
    the tile framework still emits a self-semaphore wait for it, which
    costs the producer's full completion-propagation latency on the
    consumer.  Pool (8 concurrent Q7 cores) and DMA instructions keep all
    waits.
    """
    import re
    eng_prefix = {
        mybir.EngineType.DVE: "DVE_",
        mybir.EngineType.Activation: "Activation_",
        mybir.EngineType.PE: "PE_",
    }
    n_strip = 0
    for f in nc.m.functions:
        for b in f.blocks:
            for ins in b.instructions:
                if ins.opcode in _DMA_OPS:
                    continue
                pref = eng_prefix.get(ins.engine)
                si = ins.sync_info
                if pref is None or si is None or not si.on_wait:
                    continue
                keep = [w for w in si.on_wait
                        if not (w.ant_name or "").startswith(pref)]
                if len(keep) != len(si.on_wait):
                    n_strip += len(si.on_wait) - len(keep)
                    ins.sync_info = mybir.SyncInfo(
                        on_wait=keep, on_update=list(si.on_update or []))
    return n_strip


# ---- host-side input prep ----


def _prep_mask(mask_slice, L=8):
    # mask_slice: (L, 8, 512) -> (L, 128, 256) transposed, token-repeated
    mr = np.empty((L, 128, 256), np.float32)
    for l in range(L):
        mT = mask_slice[l].T.reshape(4, 128, B).transpose(1, 0, 2)  # p,c,b
        mr[l] = np.broadcast_to(
            mT[:, :, None, :], (128, 4, 8, B)).reshape(128, 256)
    return mr


def _prep_shared(weight, bias, L=8, IN=1024):
    w_off = 0
    wx_list, wh_list, b_list = [], [], []
    for l in range(L):
        in_l = IN if l == 0 else H
        wx = weight[w_off:w_off + in_l * 3072].reshape(in_l, 3072).copy()
        w_off += in_l * 3072
        wh = weight[w_off:w_off + H * 2560].reshape(H, 2560).copy()
        w_off += H * 2560
        b = bias[l * 2560:(l + 1) * 2560].copy()
        # fold sigmoid(z) = 0.5*(1+tanh(z/2)) halving into i,f,o,r columns
        sc6 = np.ones(3072, np.float32)
        sc6[0:1024] = 0.5          # i, f
        sc6[1536:2560] = 0.5       # o, r
        wx *= sc6[None, :]
        wh *= sc6[None, :2560]
        b = b * sc6[:2560]
        wx_list.append(wx)
        wh_list.append(wh)
        b_list.append(b)

    wx0 = wx_list[0].reshape(8, 128, 3072).transpose(1, 0, 2).reshape(
        128, 8 * 3072).astype(NBF16)
    wxr = np.stack([
        wx_list[l].reshape(4, 128, 3072).transpose(1, 0, 2).reshape(
            128, 4 * 3072) for l in range(1, L)]).astype(NBF16)
    wh = np.stack([
        wh_list[l].reshape(4, 128, 2560).transpose(1, 0, 2).reshape(
            128, 4 * 2560) for l in range(L)]).astype(NBF16)
    bb = np.stack(b_list).astype(NBF16)
    return wx0, wxr, wh, bb


def _prep_x(x_slice, S):
    # x_slice: (S, B, 1024) -> (128, 8 chunks * S * B) bf16
    xt = x_slice.transpose(2, 0, 1).reshape(8, 128, S * B)
    return np.ascontiguousarray(
        xt.transpose(1, 0, 2).reshape(128, 8 * S * B)).astype(NBF16)


_CACHE = {}


def _get_nc():
    if "nc" not in _CACHE:
        nc = build_full(S=256, L=8)
        strip_self_waits(nc)
        split_excess_waits(nc, limit=2, dma_limit=2)
        _CACHE["nc"] = nc
    return _CACHE["nc"]


def hw_exec_time_estimate_ns():
    if "tl" not in _CACHE:
        from concourse.timeline_sim import TimelineSim
        _CACHE["tl"] = int(TimelineSim(_get_nc(), trace=False).simulate())
    return _CACHE["tl"]


def kernel(inputs, weight, bias, dropout_mask):
    inputs = np.ascontiguousarray(inputs, dtype=np.float32)
    weight = np.ascontiguousarray(weight, dtype=np.float32)
    bias = np.ascontiguousarray(bias, dtype=np.float32)
    dropout_mask = np.ascontiguousarray(dropout_mask, dtype=np.float32)
    S = inputs.shape[0]
    nc = _get_nc()
    wx0, wxr, wh, bb = _prep_shared(weight, bias)
    n_cores = 8
    in_maps = []
    for i in range(n_cores):
        sl = slice(B * i, B * (i + 1))
        in_maps.append({
            "xt0": _prep_x(inputs[:, sl, :], S),
            "wx0": wx0, "wxr": wxr, "whx": wh, "biasx": bb,
            "maskrep": _prep_mask(dropout_mask[:, sl, :]),
        })
    res = run_bass_kernel_spmd(nc, in_maps, list(range(n_cores)))
    outs = []
    for i in range(n_cores):
        o = res.results[i]["out"].reshape(S, 4, B, 128)
        outs.append(o.transpose(0, 2, 1, 3).reshape(S, B, H))
    out = np.concatenate(outs, axis=1)
    return np.ascontiguousarray(out, dtype=np.float32)


# revision 9
# speedup vs baseline: 1.0990x; 1.0313x over previous
"""AlternatingHighwayLSTM Trainium2 (Bass) kernel - 8-core SPMD, v2.

Fully transposed recurrence: hidden dim on partitions (4 chunks of 128),
batch (8/core) on the free dim.  Per step, z^T = Wx^T x + Wh^T h + b is
accumulated in PSUM as 24 (128, 8) chunk-slices: Wx/bias parts are
pre-accumulated per 8-step group (double-buffered 3-bank PSUM tiles),
the Wh part runs per step as 80 small bf16 matmuls (stationary = natural
weight chunks).  Sigmoid gates are computed as tanh of half-scaled
pre-activations (the 0.5 is folded into the weights host-side), so one
tanh activation covers i,f,g and one covers o,r.  The elementwise chain
uses fused scalar_tensor_tensor ops on (128, 32) tiles.

Host-side prep: weights/x are cast to bf16, chunked, and column-scaled;
everything is passed as extra DRAM inputs.

Note: like the previous version, the stored per-step state equals h*m
(exact h for the eval-mode all-ones dropout mask).
"""
import numpy as np
import ml_dtypes
import concourse.bass as bass
import concourse.mybir as mybir
import concourse.tile as tile
from concourse.bass_utils import run_bass_kernel_spmd
from concourse.masks import make_identity

F32 = mybir.dt.float32
F32R = mybir.dt.float32r
BF16 = mybir.dt.bfloat16
AF = mybir.ActivationFunctionType
OP = mybir.AluOpType

H = 512
B = 8            # per-core batch
G = 8            # steps per Wx pre-accumulation group
NBF16 = ml_dtypes.bfloat16

_DMA_OPS = {"DMACopy", "TensorLoad", "TensorSave", "DMATransposeAnt", "TriggerDMA"}


def split_excess_waits(nc, limit=1, dma_limit=1):
    n_split = 0
    uid = [0]
    for f in nc.m.functions:
        for b in f.blocks:
            out = []
            changed = False
            for ins in b.instructions:
                si = ins.sync_info
                lim = dma_limit if ins.opcode in _DMA_OPS else limit
                if si is not None and si.on_wait is not None and len(si.on_wait) > lim:
                    waits = list(si.on_wait)
                    extra, keep = waits[:-lim], waits[-lim:]
                    for w in extra:
                        ev = mybir.InstEventSemaphore(name=f"WSPLIT-{uid[0]}")
                        uid[0] += 1
                        ev.engine = ins.engine
                        ev.sync_info = mybir.SyncInfo(on_wait=[w], on_update=[])
                        out.append(ev)
                        n_split += 1
                    ins.sync_info = mybir.SyncInfo(
                        on_wait=keep, on_update=list(si.on_update or []))
                    changed = True
                out.append(ins)
            if changed:
                try:
                    b.instructions = out
                except Exception:
                    b.instructions.clear()
                    b.instructions.extend(out)
    return n_split


def build_full(S=256, L=8, dbg_hseq=False, wx_first=False, use_gpsimd=True):
    nc = bass.Bass("TRN2", target_bir_lowering=False, debug=False)
    T = S * B          # free-dim cols per hidden chunk (tokens x batch)
    NG = S // G
    dbg_d = None
    if dbg_hseq:
        dbg_d = nc.dram_tensor("dbg", [L, 128, 4 * T], BF16,
                               kind="ExternalOutput").ap()

    xt0_d = nc.dram_tensor("xt0", [128, 8 * T], BF16, kind="ExternalInput").ap()
    wx0_d = nc.dram_tensor("wx0", [128, 8 * 3072], BF16, kind="ExternalInput").ap()
    wxr_d = nc.dram_tensor("wxr", [max(L - 1, 1), 128, 4 * 3072], BF16,
                           kind="ExternalInput").ap()
    wh_d = nc.dram_tensor("whx", [L, 128, 4 * 2560], BF16,
                          kind="ExternalInput").ap()
    bias_d = nc.dram_tensor("biasx", [L, 2560], BF16, kind="ExternalInput").ap()
    mask_d = nc.dram_tensor("maskrep", [L, 128, 256], F32,
                            kind="ExternalInput").ap()
    # output layout (S, 4 chunks x 8 batch, 128): matches the PE-transpose
    # partition order; the host unshuffles to (S, B, H)
    o_d = nc.dram_tensor("out", [S, 32, 128], F32, kind="ExternalOutput").ap()

    with tile.TileContext(nc) as tc:
        with (
            tc.tile_pool(name="sb", bufs=1) as sb,
            tc.tile_pool(name="ps", bufs=1, space="PSUM") as ps,
        ):
            xT0 = sb.tile([128, 8 * T], BF16, tag="xT0")
            hseq = [sb.tile([128, 4 * T], BF16, tag=f"hseq{i}", name=f"hseq{i}")
                    for i in range(2)]
            wxs = [sb.tile([128, 4 * 3072], BF16, tag=f"wxs{i}", name=f"wxs{i}")
                   for i in range(2)]
            whs = [sb.tile([128, 4 * 2560], BF16, tag=f"whs{i}", name=f"whs{i}")
                   for i in range(2)]
            biast = [sb.tile([1, 2560], BF16, tag=f"bias{i}", name=f"bias{i}")
                     for i in range(2)]
            maskt = [sb.tile([128, 256], F32, tag=f"mask{i}", name=f"mask{i}")
                     for i in range(2)]
            ones_t = sb.tile([1, 64], BF16, tag="ones")
            gates = sb.tile([128, 192], F32, tag="gates")
            scr = sb.tile([128, 64], F32, tag="scr")
            scr2 = sb.tile([128, 64], F32, tag="scr2")
            thb = sb.tile([128, 32], F32, tag="thb")
            tln = sb.tile([128, 32], F32, tag="tln")
            Xt = sb.tile([128, 32], F32, tag="Xt")
            lin05 = [sb.tile([128, 256], F32, tag=f"lin05{i}", name=f"lin05{i}")
                     for i in range(2)]
            hm32 = sb.tile([128, 32], F32, tag="hm32")
            b1q = sb.tile([128, 32], F32, tag="b1q")
            p4q = sb.tile([128, 32], F32, tag="p4q")
            wv = sb.tile([128, 32], F32, tag="wv")
            quart = sb.tile([128, 32], F32, tag="quart")
            ob = [sb.tile([32, 128], F32, tag=f"ob{i}", name=f"ob{i}")
                  for i in range(2)]
            ident = sb.tile([128, 128], F32, tag="ident")

            # z split into two psum tiles per group buffer so act1's
            # per-tile dependency covers only i,f,g:
            #   zga = [i(0-3) f(4-7) g(8-11) lin(20-23)]  (2 banks)
            #   zgb = [o(12-15) r(16-19)]                 (1 bank)
            zga = [ps.tile([128, 1024], F32, tag=f"zga{i}", name=f"zga{i}")
                   for i in range(2)]
            zgb = [ps.tile([128, 512], F32, tag=f"zgb{i}", name=f"zgb{i}")
                   for i in range(2)]
            tp = [ps.tile([128, 512], F32, tag=f"tp{i}", name=f"tp{i}")
                  for i in range(2)]

            def zslice(g, c):
                # -> (tile, column offset) for chunk c of group g
                if c < 12:
                    return zga[g % 2], c * 64
                if c < 20:
                    return zgb[g % 2], (c - 12) * 64
                return zga[g % 2], 768 + (c - 20) * 64

            make_identity(nc, ident[:, :])
            nc.vector.memset(ones_t[0:1, :], 1.0)
            nc.vector.memset(quart[:, :], 0.25)

            # initial loads
            nc.sync.dma_start(out=xT0[:, :], in_=xt0_d)
            nc.sync.dma_start(out=wxs[0][:, :], in_=wx0_d[:, 0:12288])
            nc.sync.dma_start(out=wxs[1][:, :], in_=wx0_d[:, 12288:24576])
            nc.sync.dma_start(out=whs[0][:, :], in_=wh_d[0])
            nc.sync.dma_start(out=biast[0][0:1, :],
                              in_=bias_d[0].rearrange("(o c) -> o c", o=1))
            nc.sync.dma_start(out=maskt[0][:, :], in_=mask_d[0])

            for l in range(L):
                even = (l % 2 == 0)
                ind = l % 2
                xin = xT0 if l == 0 else hseq[(l - 1) % 2]
                KCX = 8 if l == 0 else 4
                whv = whs[l % 2]
                bv = biast[l % 2]
                mkt = maskt[l % 2]

                def wxv(k, l=l):
                    if l == 0:
                        return wxs[k // 4][:, (k % 4) * 3072:(k % 4 + 1) * 3072]
                    return wxs[(l + 1) % 2][:, k * 3072:(k + 1) * 3072]

                def tok(s, even=even):
                    return s if even else S - 1 - s

                def tokbase(g, even=even):
                    return g * G if even else S - (g + 1) * G

                def wx_group_thunks(g, xin=xin, KCX=KCX, wxv=wxv,
                                    tokbase=tokbase, bv=bv):
                    tb = tokbase(g)
                    # psum banks: A0 = c0-7, A1 = c8-11 + c20-23, B0 = c12-19
                    def bank(c):
                        return 0 if c < 8 else (1 if c < 12 or c >= 20 else 2)
                    items = [("b", 0), ("b", 8), ("b", 12)]
                    for c in range(24):
                        if c < 20 and c not in (0, 8, 12):
                            items.append(("b", c))
                        for k in range(KCX):
                            items.append(("x", c, k))
                    first = {0: True, 1: True, 2: True}
                    thunks = []
                    for it in items:
                        st = first[bank(it[1])]
                        first[bank(it[1])] = False
                        if it[0] == "b":
                            c = it[1]

                            def t(c=c, st=st, g=g):
                                zt, co = zslice(g, c)
                                nc.tensor.matmul(
                                    zt[:, co:co + 64],
                                    bv[0:1, c * 128:(c + 1) * 128],
                                    ones_t[0:1, 0:64],
                                    start=st, stop=False, skip_group_check=True)
                        else:
                            c, k = it[1], it[2]

                            def t(c=c, k=k, st=st, g=g, tb=tb):
                                zt, co = zslice(g, c)
                                nc.tensor.matmul(
                                    zt[:, co:co + 64],
                                    wxv(k)[:, c * 128:(c + 1) * 128],
                                    xin[:, k * T + tb * 8:k * T + (tb + G) * 8],
                                    start=st, stop=False, skip_group_check=True)
                        thunks.append(t)
                    return thunks

                # group 0 burst
                for t in wx_group_thunks(0):
                    t()

                # prefetch next layer's weights (the wx slot for layer l+1 is
                # only free of emitted readers once layer l-1 is emitted; for
                # l==0 both slots are read by layer 0 itself, so its wx
                # prefetch is emitted after the step loop below)
                if l + 1 < L:
                    if l > 0:
                        nc.sync.dma_start(out=wxs[(l + 2) % 2][:, :],
                                          in_=wxr_d[l])
                    nc.sync.dma_start(out=whs[(l + 1) % 2][:, :], in_=wh_d[l + 1])
                    nc.sync.dma_start(
                        out=biast[(l + 1) % 2][0:1, :],
                        in_=bias_d[l + 1].rearrange("(o c) -> o c", o=1))
                    nc.sync.dma_start(out=maskt[(l + 1) % 2][:, :],
                                      in_=mask_d[l + 1])

                nc.vector.memset(gates[:, 96:128], 0.0)  # c-hat = 0

                pending = []
                for s in range(S):
                    g, j = divmod(s, G)
                    zA3 = zga[g % 2].rearrange("p (c x) -> p c x", c=16)
                    zB3 = zgb[g % 2].rearrange("p (c x) -> p c x", c=8)
                    pos = j if even else G - 1 - j
                    zoff = pos * 8

                    if j == 0:
                        if g + 1 < NG:
                            pending = wx_group_thunks(g + 1)
                        else:
                            pending = []
                        if s == 0:
                            # group 0: lin cols just produced by the burst
                            nc.vector.scalar_tensor_tensor(
                                out=lin05[0][:, :],
                                in0=zga[0][:, 768:1024],
                                scalar=0.5, in1=mkt[:, :],
                                op0=OP.mult, op1=OP.mult)

                    # optionally emit next group's Wx slice before the Wh
                    # burst (they have no step-chain deps)
                    if wx_first and pending:
                        nsl = (len(pending) + (G - 1 - j)) // (G - j)
                        for t in pending[:nsl]:
                            t()
                        pending = pending[nsl:]

                    # Wh matmuls for this step (rhs = previous step's hm)
                    if s > 0:
                        tp_ = tok(s - 1)
                        for c in range(20):
                            for k in range(4):
                                stop = (j == G - 1 and k == 3
                                        and c in (7, 11, 19))
                                zt, co = zslice(g, c)
                                nc.tensor.matmul(
                                    zt[:, co + zoff:co + zoff + 8],
                                    whv[:, k * 2560 + c * 128:
                                        k * 2560 + (c + 1) * 128],
                                    hseq[ind][:, k * T + tp_ * 8:
                                              k * T + tp_ * 8 + 8],
                                    start=False, stop=stop,
                                    skip_group_check=True)

                    # activations: tanh(z') with 0.5 folded into weights for
                    # sigmoid gates -> [ti tf tg | to tr]
                    nc.scalar.activation(
                        gates[:, 0:96].rearrange("p (c x) -> p c x", c=12),
                        zA3[:, 0:12, zoff:zoff + 8], AF.Tanh)
                    nc.scalar.activation(
                        gates[:, 128:192].rearrange("p (c x) -> p c x", c=8),
                        zB3[:, 0:8, zoff:zoff + 8], AF.Tanh)

                    # [t2|u2] = ([ti|tf] + 1) * [tg|chat]
                    nc.vector.scalar_tensor_tensor(
                        out=scr[:, 0:64], in0=gates[:, 0:64], scalar=1.0,
                        in1=gates[:, 64:128], op0=OP.add, op1=OP.mult)
                    # chat' = 0.5*u2 + t2   (chat = 2c)
                    nc.vector.scalar_tensor_tensor(
                        out=gates[:, 96:128], in0=scr[:, 32:64], scalar=0.5,
                        in1=scr[:, 0:32], op0=OP.mult, op1=OP.add)

                    l5v = lin05[g % 2].rearrange("p (c x) -> p c x", c=4)[
                        :, :, zoff:zoff + 8]
                    trv = gates[:, 160:192].rearrange("p (c x) -> p c x", c=4)
                    # off-path: p4q = r*o = 0.25*(to+1)*(tr+1)  (queued
                    # before X so the Pool-gated X doesn't delay it)
                    nc.vector.scalar_tensor_tensor(
                        out=b1q[:, :], in0=gates[:, 128:160], scalar=1.0,
                        in1=quart[:, :], op0=OP.add, op1=OP.mult)
                    nc.vector.scalar_tensor_tensor(
                        out=p4q[:, :], in0=gates[:, 160:192], scalar=1.0,
                        in1=b1q[:, :], op0=OP.add, op1=OP.mult)
                    # tlnm = tr * lin05m ; X = lin05m - tlnm = (1-r)*lin*m
                    # (Pool supports TensorTensor but not TensorScalarPtr)
                    veng = nc.gpsimd if use_gpsimd else nc.vector
                    veng.tensor_tensor(
                        tln[:, :].rearrange("p (c x) -> p c x", c=4),
                        trv, l5v, OP.mult)
                    nc.vector.scalar_tensor_tensor(
                        out=Xt[:, :].rearrange("p (c x) -> p c x", c=4),
                        in0=tln[:, :].rearrange("p (c x) -> p c x", c=4),
                        scalar=-1.0, in1=l5v, op0=OP.mult, op1=OP.add)
                    # th = tanh(c) = tanh(0.5 * chat)
                    nc.scalar.activation(thb[:, :], gates[:, 96:128], AF.Tanh,
                                         scale=0.5)
                    # w = r*o*th ; hm = w + X   (short post-tanh path)
                    nc.vector.tensor_tensor(wv[:, :], p4q[:, :], thb[:, :],
                                            OP.mult)
                    # hm -> hseq (bf16, strided chunk view)
                    hv = hseq[ind].rearrange("p (k t) -> p k t", k=4)[
                        :, :, tok(s) * 8:tok(s) * 8 + 8]
                    nc.vector.tensor_tensor(
                        hv, wv[:, :].rearrange("p (c x) -> p c x", c=4),
                        Xt[:, :].rearrange("p (c x) -> p c x", c=4), OP.add)

                    if l == L - 1:
                        nc.vector.tensor_tensor(hm32[:, :], wv[:, :],
                                                Xt[:, :], OP.add)
                        nc.tensor.transpose(
                            tp[s % 2][0:32, 0:128],
                            hm32[:, 0:32], ident[:, :])
                        nc.scalar.copy(ob[s % 2][0:32, :],
                                       tp[s % 2][0:32, 0:128])
                        nc.sync.dma_start(out=o_d[tok(s), :, :],
                                          in_=ob[s % 2][0:32, :])

                    if dbg_hseq and s == S - 1:
                        nc.sync.dma_start(out=dbg_d[l], in_=hseq[ind][:, :])

                    # spread next group's Wx/bias matmuls
                    if pending and not wx_first:
                        nsl = (len(pending) + (G - 1 - j)) // (G - j)
                        for t in pending[:nsl]:
                            t()
                        pending = pending[nsl:]

                    if j == G - 1 and g + 1 < NG:
                        # next group's lin05m at the tail of this step's DVE
                        # queue (after its Wx lin matmuls are all emitted)
                        nc.vector.scalar_tensor_tensor(
                            out=lin05[(g + 1) % 2][:, :],
                            in0=zga[(g + 1) % 2][:, 768:1024],
                            scalar=0.5, in1=mkt[:, :],
                            op0=OP.mult, op1=OP.mult)

                if l == 0 and L > 1:
                    nc.sync.dma_start(out=wxs[0][:, :], in_=wxr_d[0])
    return nc


def strip_self_waits(nc):
    """Remove waits on an instruction's own engine-completion semaphore.

    DVE/Activation/PE engine queues execute strictly in order, so a
    same-engine RAW/WAR dependency is already satisfied by program order;
    the tile framework still emits a self-semaphore wait for it, which
    costs the producer's full completion-propagation latency on the
    consumer.  Pool (8 concurrent Q7 cores) and DMA instructions keep all
    waits.
    """
    import re
    eng_prefix = {
        mybir.EngineType.DVE: "DVE_",
        mybir.EngineType.Activation: "Activation_",
        mybir.EngineType.PE: "PE_",
    }
    n_strip = 0
    for f in nc.m.functions:
        for b in f.blocks:
            for ins in b.instructions:
                if ins.opcode in _DMA_OPS:
                    continue
                pref = eng_prefix.get(ins.engine)
                si = ins.sync_info
                if pref is None or si is None or not si.on_wait:
                    continue
                keep = [w for w in si.on_wait
                        if not (w.ant_name or "").startswith(pref)]
                if len(keep) != len(si.on_wait):
                    n_strip += len(si.on_wait) - len(keep)
                    ins.sync_info = mybir.SyncInfo(
                        on_wait=keep, on_update=list(si.on_update or []))
    return n_strip


# ---- host-side input prep ----


def _prep_mask(mask_slice, L=8):
    # mask_slice: (L, 8, 512) -> (L, 128, 256) transposed, token-repeated
    mr = np.empty((L, 128, 256), np.float32)
    for l in range(L):
        mT = mask_slice[l].T.reshape(4, 128, B).transpose(1, 0, 2)  # p,c,b
        mr[l] = np.broadcast_to(
            mT[:, :, None, :], (128, 4, 8, B)).reshape(128, 256)
    return mr


def _prep_shared(weight, bias, L=8, IN=1024):
    w_off = 0
    wx_list, wh_list, b_list = [], [], []
    for l in range(L):
        in_l = IN if l == 0 else H
        wx = weight[w_off:w_off + in_l * 3072].reshape(in_l, 3072).copy()
        w_off += in_l * 3072
        wh = weight[w_off:w_off + H * 2560].reshape(H, 2560).copy()
        w_off += H * 2560
        b = bias[l * 2560:(l + 1) * 2560].copy()
        # fold sigmoid(z) = 0.5*(1+tanh(z/2)) halving into i,f,o,r columns
        sc6 = np.ones(3072, np.float32)
        sc6[0:1024] = 0.5          # i, f
        sc6[1536:2560] = 0.5       # o, r
        wx *= sc6[None, :]
        wh *= sc6[None, :2560]
        b = b * sc6[:2560]
        wx_list.append(wx)
        wh_list.append(wh)
        b_list.append(b)

    wx0 = wx_list[0].reshape(8, 128, 3072).transpose(1, 0, 2).reshape(
        128, 8 * 3072).astype(NBF16)
    wxr = np.stack([
        wx_list[l].reshape(4, 128, 3072).transpose(1, 0, 2).reshape(
            128, 4 * 3072) for l in range(1, L)]).astype(NBF16)
    wh = np.stack([
        wh_list[l].reshape(4, 128, 2560).transpose(1, 0, 2).reshape(
            128, 4 * 2560) for l in range(L)]).astype(NBF16)
    bb = np.stack(b_list).astype(NBF16)
    return wx0, wxr, wh, bb


def _prep_x(x_slice, S):
    # x_slice: (S, B, 1024) -> (128, 8 chunks * S * B) bf16
    xt = x_slice.transpose(2, 0, 1).reshape(8, 128, S * B)
    return np.ascontiguousarray(
        xt.transpose(1, 0, 2).reshape(128, 8 * S * B)).astype(NBF16)


_CACHE = {}


def _get_nc():
    if "nc" not in _CACHE:
        nc = build_full(S=256, L=8)
        strip_self_waits(nc)
        split_excess_waits(nc)
        _CACHE["nc"] = nc
    return _CACHE["nc"]


def hw_exec_time_estimate_ns():
    if "tl" not in _CACHE:
        from concourse.timeline_sim import TimelineSim
        _CACHE["tl"] = int(TimelineSim(_get_nc(), trace=False).simulate())
    return _CACHE["tl"]


def kernel(inputs, weight, bias, dropout_mask):
    inputs = np.ascontiguousarray(inputs, dtype=np.float32)
    weight = np.ascontiguousarray(weight, dtype=np.float32)
    bias = np.ascontiguousarray(bias, dtype=np.float32)
    dropout_mask = np.ascontiguousarray(dropout_mask, dtype=np.float32)
    S = inputs.shape[0]
    nc = _get_nc()
    wx0, wxr, wh, bb = _prep_shared(weight, bias)
    n_cores = 8
    in_maps = []
    for i in range(n_cores):
        sl = slice(B * i, B * (i + 1))
        in_maps.append({
            "xt0": _prep_x(inputs[:, sl, :], S),
            "wx0": wx0, "wxr": wxr, "whx": wh, "biasx": bb,
            "maskrep": _prep_mask(dropout_mask[:, sl, :]),
        })
    res = run_bass_kernel_spmd(nc, in_maps, list(range(n_cores)))
    outs = []
    for i in range(n_cores):
        o = res.results[i]["out"].reshape(S, 4, B, 128)
        outs.append(o.transpose(0, 2, 1, 3).reshape(S, B, H))
    out = np.concatenate(outs, axis=1)
    return np.ascontiguousarray(out, dtype=np.float32)
